# revision 10
# baseline (speedup 1.0000x reference)
"""Trainium2 Bass kernel for nn_ContrastiveLoss (N=M=8192, D=768, 16 labels).

Strategy (8 NeuronCores, SPMD, no collectives):
  - Row-stripe sharding: core c owns rows [1024c, 1024(c+1)) of joint_embeddings.
  - Each core computes its [1024, 8192] block of BOTH distance matrices
    (joint-vs-joint and joint-vs-non-joint) as tiled bf16 matmuls on the PE:
        d2[i,j] = sx[i] + sx[j] - 2*g[i,j] + D*eps^2   (g = x_i . x_j)
    The label-equality mask is folded into the matmul as 16 extra one-hot
    contraction rows contributing +BIG*same[i,j]; the |x_j|^2 row rides along
    as three extra bf16 rows (hi/mid/lo split, ~24 mantissa bits); |x_i|^2
    enters via the ACT bias.  The masked positive sum then falls out of a
    single fused Relu+row-sum on the Scalar engine:
        pos += sum relu(d2 + BIG*same - BIG)       (diff pairs killed by -BIG)
  - The hinge terms relu(margin - dist)^2 are zero unless d2 < margin^2 = 1.
    For every tile we also accumulate the exact trigger mass
        guard = sum relu(1 - (d2 + BIG*same))      (jj: diff pairs only)
        guard = sum relu(1 - d2)                   (jn: all pairs)
    which is 0 iff no pair is inside the margin.  If any guard fires (never
    for data in this regime: pair distances concentrate around sqrt(2D) ~ 39),
    the host falls back to an exact numpy evaluation.
  - Embeddings are converted to bf16 on-device, staged to DRAM, and the
    transposed [d, j] operand layout is produced by a handful of large
    DRAM->SBUF DMA-transposes (the xbar path needs 2-byte dtype).
  - Host combines 8x[128,32] partial-sum tiles in float64.

Upper-triangle restriction of the jj matrix is handled by symmetry: the full
off-diagonal same-label sum is exactly twice the i<j sum (the antisymmetric
2*eps*(rx_i - rx_j) cross term cancels in the pair sum; its contribution to
the reference's upper sum is ~5e-11 relative and is dropped).
"""

import numpy as np

N = 8192
D = 768
N_CORES = 8
CORE_ROWS = N // N_CORES          # 1024
PANEL = 512
N_PANELS = N // PANEL             # 16
QCOLS = 2048                      # columns per transpose quarter / PSUM group
NQ = N // QCOLS                   # 4
QPANELS = QCOLS // PANEL          # 4
QTILES = QCOLS // 128             # 16 natural row-tiles per quarter
KT = D // 128                     # 6 contraction tiles
TI = CORE_ROWS // 128             # 8 i-tiles per core
NSLOTS = TI * NQ                  # 32 accum slots per phase

BIG = 32768.0
EPS = 1e-6
D_EPS2 = D * EPS * EPS
MARGIN = 1.0
LOSS_WEIGHT = 1.0
N_LABELS = 16
EXROWS = 3 + N_LABELS             # b_hi, b_mid, b_lo, 16 one-hot rows

_CACHE = {}


def _build_program():
    import concourse.bacc as bacc
    import concourse.tile as tile
    from concourse import mybir

    f32 = mybir.dt.float32
    bf16 = mybir.dt.bfloat16
    Alu = mybir.AluOpType
    Act = mybir.ActivationFunctionType

    nc = bacc.Bacc("TRN2", target_bir_lowering=False, debug=False,
                   num_devices=N_CORES)

    xfull = nc.declare_dram_parameter("xfull", [N, D], f32, isOutput=False)
    yfull = nc.declare_dram_parameter("yfull", [N, D], f32, isOutput=False)
    xc = nc.declare_dram_parameter("xc", [CORE_ROWS, D], f32, isOutput=False)
    ohb = nc.declare_dram_parameter("ohb", [N_LABELS, N], bf16, isOutput=False)
    exs = nc.declare_dram_parameter("exs", [EXROWS, CORE_ROWS], bf16,
                                    isOutput=False)
    pos_out = nc.declare_dram_parameter("pos_out", [128, NSLOTS], f32,
                                        isOutput=True)
    gjj_out = nc.declare_dram_parameter("gjj_out", [128, NSLOTS], f32,
                                        isOutput=True)
    gjn_out = nc.declare_dram_parameter("gjn_out", [128, NSLOTS], f32,
                                        isOutput=True)

    with tile.TileContext(nc) as tc:
        with (
            tc.tile_pool(name="singles", bufs=1) as singles,
            tc.tile_pool(name="dram", bufs=1, space="DRAM") as dramp,
            tc.tile_pool(name="nat", bufs=8) as natp,
            tc.tile_pool(name="natb", bufs=8) as natbp,
            tc.tile_pool(name="qt", bufs=2) as qtp,
            tc.tile_pool(name="extram", bufs=8) as extramp,
            tc.tile_pool(name="sqscr", bufs=3) as sqscrp,
            tc.tile_pool(name="trash", bufs=3) as trashp,
            tc.tile_pool(name="smalls", bufs=4) as smallp,
            tc.tile_pool(name="psum", bufs=2, space="PSUM") as psump,
        ):
            # ---- persistent tiles ----
            statT = singles.tile([128, KT, CORE_ROWS], bf16)   # -2 * xc^T
            sxc = singles.tile([128, TI], f32)
            bias_pos = singles.tile([128, TI], f32)
            bias_g = singles.tile([128, TI], f32)
            pos_acc = singles.tile([128, NSLOTS], f32)
            gjj_acc = singles.tile([128, NSLOTS], f32)
            gjn_acc = singles.tile([128, NSLOTS], f32)
            extraS = singles.tile([EXROWS, TI, 128], bf16)

            nc.gpsimd.dma_start(
                out=extraS[:, :, :],
                in_=exs[:, :].rearrange("c (t i) -> c t i", t=TI))

            # ---- phase 0: stationary = -2 * x_c^T (bf16) + own-row biases ----
            sbf = dramp.tile([CORE_ROWS, D], bf16, tag="sbf")
            for b in range(TI):
                nat = natp.tile([128, D], f32, tag="nat")
                nc.gpsimd.dma_start(out=nat, in_=xc[128 * b:128 * (b + 1), :])
                natb = natbp.tile([128, D], bf16, tag="natb")
                nc.vector.tensor_scalar_mul(out=natb, in0=nat, scalar1=-2.0)
                sq = sqscrp.tile([128, D], f32, tag="sq")
                nc.vector.scalar_tensor_tensor(
                    out=sq, in0=natb, scalar=0.25, in1=natb,
                    op0=Alu.mult, op1=Alu.mult, accum_out=sxc[:, b:b + 1])
                nc.gpsimd.dma_start(out=sbf[128 * b:128 * (b + 1), :], in_=natb)
            for kt in range(KT):
                nc.sync.dma_start_transpose(
                    out=statT[:, kt, :],
                    in_=sbf[:, 128 * kt:128 * (kt + 1)])

            # pos wants relu(psum + a_i - BIG); guard wants relu(-psum + 1 - a_i)
            nc.vector.tensor_scalar(
                out=bias_pos, in0=sxc, scalar1=float(D_EPS2 - BIG),
                scalar2=None, op0=Alu.add)
            nc.vector.tensor_scalar(
                out=bias_g, in0=sxc, scalar1=-1.0,
                scalar2=float(1.0 - D_EPS2), op0=Alu.mult, op1=Alu.add)

            # ---- conversion sweep: fp32 -> bf16 DRAM + |x_j|^2 rows ----
            def convert_quarter(src, mq, qi):
                """Convert 2048 rows to bf16 DRAM; build the [3,16,128] bf16
                hi/mid/lo staged rows of |x_j|^2."""
                qbf = dramp.tile([QCOLS, D], bf16, tag=f"qbf{mq}{qi}")
                qsx = smallp.tile([128, QTILES], f32, tag="qsx")
                for i in range(QTILES):
                    nat = natp.tile([128, D], f32, tag="nat")
                    r0 = QCOLS * qi + 128 * i
                    nc.gpsimd.dma_start(out=nat, in_=src[r0:r0 + 128, :])
                    natb = natbp.tile([128, D], bf16, tag="natb")
                    nc.vector.tensor_copy(out=natb, in_=nat)
                    sq = sqscrp.tile([128, D], f32, tag="sq")
                    nc.vector.scalar_tensor_tensor(
                        out=sq, in0=natb, scalar=1.0, in1=natb,
                        op0=Alu.mult, op1=Alu.mult,
                        accum_out=qsx[:, i:i + 1])
                    nc.scalar.dma_start(
                        out=qbf[128 * i:128 * (i + 1), :], in_=natb)
                # 3-way bf16 split of the f32 row sums
                hi = smallp.tile([128, QTILES], bf16, tag="hi")
                mid = smallp.tile([128, QTILES], bf16, tag="mid")
                lo = smallp.tile([128, QTILES], bf16, tag="lo")
                r1 = smallp.tile([128, QTILES], f32, tag="r1")
                r2 = smallp.tile([128, QTILES], f32, tag="r2")
                nc.vector.tensor_copy(out=hi, in_=qsx)
                nc.vector.tensor_tensor(out=r1, in0=qsx, in1=hi,
                                        op=Alu.subtract)
                nc.vector.tensor_copy(out=mid, in_=r1)
                nc.vector.tensor_tensor(out=r2, in0=r1, in1=mid,
                                        op=Alu.subtract)
                nc.vector.tensor_copy(out=lo, in_=r2)
                stg = dramp.tile([3, QTILES, 128], bf16, tag=f"stg{mq}{qi}")
                nc.gpsimd.dma_start(
                    out=stg[0, :, :].rearrange("f p -> p f"), in_=hi)
                nc.gpsimd.dma_start(
                    out=stg[1, :, :].rearrange("f p -> p f"), in_=mid)
                nc.gpsimd.dma_start(
                    out=stg[2, :, :].rearrange("f p -> p f"), in_=lo)
                return qbf, stg

            # ---- main sweep: convert quarter, then immediately compute on
            # it, so engine streams interleave and PSUM drains promptly ----
            for phase, mq, src in (("jj", "x", xfull), ("jn", "y", yfull)):
                for qi in range(NQ):
                    qbf, stg = convert_quarter(src, mq, qi)
                    qt = qtp.tile([128, KT, QCOLS], bf16, tag="qt")
                    for kt in range(KT):
                        nc.sync.dma_start_transpose(
                            out=qt[:, kt, :],
                            in_=qbf[:, 128 * kt:128 * (kt + 1)])
                    ems = []
                    for pq in range(QPANELS):
                        em = extramp.tile([EXROWS, PANEL], bf16, tag="em")
                        nc.gpsimd.dma_start(
                            out=em[0:3, :],
                            in_=stg[:, 4 * pq:4 * (pq + 1), :].rearrange(
                                "c f p -> c (f p)"))
                        if phase == "jj":
                            p = QPANELS * qi + pq
                            nc.gpsimd.dma_start(
                                out=em[3:EXROWS, :],
                                in_=ohb[:, PANEL * p:PANEL * (p + 1)])
                        ems.append(em)
                    for t in range(TI):
                        psum = psump.tile([128, QCOLS], f32, tag="ps")
                        for pq in range(QPANELS):
                            col = slice(PANEL * pq, PANEL * (pq + 1))
                            for kt in range(KT):
                                nc.tensor.matmul(
                                    out=psum[:, col],
                                    lhsT=statT[:, kt, 128 * t:128 * (t + 1)],
                                    rhs=qt[:, kt, col],
                                    start=(kt == 0), stop=False)
                            nrows = EXROWS if phase == "jj" else 3
                            nc.tensor.matmul(
                                out=psum[:, col],
                                lhsT=extraS[0:nrows, t, :],
                                rhs=ems[pq][0:nrows, :],
                                start=False, stop=True)
                        s = t * NQ + qi
                        if phase == "jj":
                            tr = trashp.tile([128, QCOLS], f32, tag="tr")
                            nc.scalar.activation(
                                out=tr, in_=psum, func=Act.Relu,
                                bias=bias_pos[:, t:t + 1], scale=1.0,
                                accum_out=pos_acc[:, s:s + 1])
                            tr2 = trashp.tile([128, QCOLS], f32, tag="tr")
                            nc.scalar.activation(
                                out=tr2, in_=psum, func=Act.Relu,
                                bias=bias_g[:, t:t + 1], scale=-1.0,
                                accum_out=gjj_acc[:, s:s + 1])
                        else:
                            tr = trashp.tile([128, QCOLS], f32, tag="tr")
                            nc.scalar.activation(
                                out=tr, in_=psum, func=Act.Relu,
                                bias=bias_g[:, t:t + 1], scale=-1.0,
                                accum_out=gjn_acc[:, s:s + 1])

            nc.gpsimd.dma_start(out=pos_out[:, :], in_=pos_acc)
            nc.gpsimd.dma_start(out=gjj_out[:, :], in_=gjj_acc)
            nc.gpsimd.dma_start(out=gjn_out[:, :], in_=gjn_acc)

    nc.compile()
    return nc


def _get_program():
    if "nc" not in _CACHE:
        _CACHE["nc"] = _build_program()
    return _CACHE["nc"]


def _host_inputs(joint_embeddings, non_joint_embeddings, joint_labels):
    import ml_dtypes

    x = np.ascontiguousarray(joint_embeddings, dtype=np.float32)
    y = np.ascontiguousarray(non_joint_embeddings, dtype=np.float32)
    lab = np.asarray(joint_labels).astype(np.int64)
    onehot = (lab[None, :] == np.arange(N_LABELS, dtype=np.int64)[:, None])
    ohb = (onehot.astype(np.float32) * np.float32(BIG)).astype(
        ml_dtypes.bfloat16)
    in_maps = []
    for c in range(N_CORES):
        rows = slice(CORE_ROWS * c, CORE_ROWS * (c + 1))
        exs = np.concatenate(
            [np.ones((3, CORE_ROWS), dtype=np.float32),
             onehot[:, rows].astype(np.float32)], axis=0).astype(
                 ml_dtypes.bfloat16)
        in_maps.append({
            "xfull": x, "yfull": y,
            "xc": np.ascontiguousarray(x[rows]),
            "ohb": ohb, "exs": np.ascontiguousarray(exs),
        })
    return in_maps, lab


def _fallback_numpy(x, y, lab):
    """Exact reference evaluation (float64), chunked. Only used when a
    guard fired, i.e. some pair distance is inside the margin."""
    x = x.astype(np.float64)
    y = y.astype(np.float64)
    sx = (x * x).sum(1)
    sy = (y * y).sum(1)
    rx = x.sum(1)
    ry = y.sum(1)
    n = x.shape[0]
    pos_sum = 0.0
    neg_sum = 0.0
    cross_sum = 0.0
    same = lab[:, None] == lab[None, :]
    for i0 in range(0, n, 512):
        i1 = min(i0 + 512, n)
        g = x[i0:i1] @ x.T
        d2 = (sx[i0:i1, None] + sx[None, :] - 2 * g
              + 2 * EPS * (rx[i0:i1, None] - rx[None, :]) + D_EPS2)
        d2 = np.maximum(d2, 0.0)
        upper = np.arange(n)[None, :] > np.arange(i0, i1)[:, None]
        sm = same[i0:i1]
        pos_sum += d2[upper & sm].sum()
        dist = np.sqrt(np.maximum(d2, 1e-12))
        t = np.maximum(MARGIN - dist, 0.0) ** 2
        neg_sum += t[upper & ~sm].sum()
        gy = x[i0:i1] @ y.T
        d2y = (sx[i0:i1, None] + sy[None, :] - 2 * gy
               + 2 * EPS * (rx[i0:i1, None] - ry[None, :]) + D_EPS2)
        d2y = np.maximum(d2y, 0.0)
        disty = np.sqrt(np.maximum(d2y, 1e-12))
        cross_sum += (np.maximum(MARGIN - disty, 0.0) ** 2).sum()
    counts = np.bincount(lab, minlength=N_LABELS)
    n_pos = max(int((counts * (counts - 1) // 2).sum()), 1)
    n_neg = max(n * (n - 1) // 2 - int((counts * (counts - 1) // 2).sum()), 1)
    loss = (pos_sum / n_pos + neg_sum / n_neg
            + cross_sum / (x.shape[0] * y.shape[0]))
    return np.float32(LOSS_WEIGHT * loss)


def kernel(joint_embeddings, non_joint_embeddings, joint_labels):
    from concourse.bass_utils import run_bass_kernel_spmd

    nc = _get_program()
    in_maps, lab = _host_inputs(joint_embeddings, non_joint_embeddings,
                                joint_labels)
    res = run_bass_kernel_spmd(nc, in_maps, core_ids=list(range(N_CORES)))
    _CACHE["last_results"] = res
    return _combine(res.results, joint_embeddings, non_joint_embeddings, lab)


def _combine(results, joint_embeddings, non_joint_embeddings, lab):
    pos_full = 0.0
    guard = 0.0
    for r in results:
        pos_full += float(r["pos_out"].astype(np.float64).sum())
        guard += float(r["gjj_out"].astype(np.float64).sum())
        guard += float(r["gjn_out"].astype(np.float64).sum())
    if guard > 0.0:
        return _fallback_numpy(
            np.asarray(joint_embeddings, dtype=np.float32),
            np.asarray(non_joint_embeddings, dtype=np.float32), lab)
    counts = np.bincount(lab, minlength=N_LABELS)
    n_pos = max(int((counts * (counts - 1) // 2).sum()), 1)
    loss = pos_full / 2.0 / n_pos
    return np.float32(LOSS_WEIGHT * loss)


# revision 11
# speedup vs baseline: 1.0994x; 1.0994x over previous
"""Trainium2 Bass kernel for nn_ContrastiveLoss (N=M=8192, D=768, 16 labels).

Strategy (8 NeuronCores, SPMD, no collectives):
  - Row-stripe sharding: core c owns rows [1024c, 1024(c+1)) of joint_embeddings.
  - Each core computes its [1024, 8192] block of BOTH distance matrices
    (joint-vs-joint and joint-vs-non-joint) as tiled bf16 matmuls on the PE:
        d2[i,j] = sx[i] + sx[j] - 2*g[i,j] + D*eps^2   (g = x_i . x_j)
    The label-equality mask is folded into the matmul as 16 extra one-hot
    contraction rows contributing +BIG*same[i,j]; the |x_j|^2 row rides along
    as three extra bf16 rows (hi/mid/lo split, ~24 mantissa bits); |x_i|^2
    enters via the ACT bias.  The masked positive sum then falls out of a
    single fused Relu+row-sum on the Scalar engine:
        pos += sum relu(d2 + BIG*same - BIG)       (diff pairs killed by -BIG)
  - The hinge terms relu(margin - dist)^2 are zero unless d2 < margin^2 = 1.
    For every tile we also accumulate the exact trigger mass
        guard = sum relu(1 - (d2 + BIG*same))      (jj: diff pairs only)
        guard = sum relu(1 - d2)                   (jn: all pairs)
    which is 0 iff no pair is inside the margin.  If any guard fires (never
    for data in this regime: pair distances concentrate around sqrt(2D) ~ 39),
    the host falls back to an exact numpy evaluation.
  - Embeddings are converted to bf16 on-device, staged to DRAM, and the
    transposed [d, j] operand layout is produced by a handful of large
    DRAM->SBUF DMA-transposes (the xbar path needs 2-byte dtype).
  - Host combines 8x[128,32] partial-sum tiles in float64.

Upper-triangle restriction of the jj matrix is handled by symmetry: the full
off-diagonal same-label sum is exactly twice the i<j sum (the antisymmetric
2*eps*(rx_i - rx_j) cross term cancels in the pair sum; its contribution to
the reference's upper sum is ~5e-11 relative and is dropped).
"""

import numpy as np

N = 8192
D = 768
N_CORES = 8
CORE_ROWS = N // N_CORES          # 1024
PANEL = 512
N_PANELS = N // PANEL             # 16
QCOLS = 2048                      # columns per transpose quarter / PSUM group
NQ = N // QCOLS                   # 4
QPANELS = QCOLS // PANEL          # 4
QTILES = QCOLS // 128             # 16 natural row-tiles per quarter
KT = D // 128                     # 6 contraction tiles
TI = CORE_ROWS // 128             # 8 i-tiles per core
NSLOTS = TI * NQ                  # 32 accum slots per phase

BIG = 32768.0
EPS = 1e-6
D_EPS2 = D * EPS * EPS
MARGIN = 1.0
LOSS_WEIGHT = 1.0
N_LABELS = 16
EXROWS = 3 + N_LABELS             # b_hi, b_mid, b_lo, 16 one-hot rows

_CACHE = {}


def _build_program():
    import concourse.bacc as bacc
    import concourse.tile as tile
    from concourse import mybir

    f32 = mybir.dt.float32
    bf16 = mybir.dt.bfloat16
    Alu = mybir.AluOpType
    Act = mybir.ActivationFunctionType

    nc = bacc.Bacc("TRN2", target_bir_lowering=False, debug=False,
                   num_devices=N_CORES)

    xfull = nc.declare_dram_parameter("xfull", [N, D], f32, isOutput=False)
    yfull = nc.declare_dram_parameter("yfull", [N, D], f32, isOutput=False)
    xc = nc.declare_dram_parameter("xc", [CORE_ROWS, D], f32, isOutput=False)
    ohb = nc.declare_dram_parameter("ohb", [N_LABELS, N], bf16, isOutput=False)
    exs = nc.declare_dram_parameter("exs", [EXROWS, CORE_ROWS], bf16,
                                    isOutput=False)
    pos_out = nc.declare_dram_parameter("pos_out", [128, NSLOTS], f32,
                                        isOutput=True)
    gjj_out = nc.declare_dram_parameter("gjj_out", [128, NSLOTS], f32,
                                        isOutput=True)
    gjn_out = nc.declare_dram_parameter("gjn_out", [128, NSLOTS], f32,
                                        isOutput=True)

    with tile.TileContext(nc) as tc:
        with (
            tc.tile_pool(name="singles", bufs=1) as singles,
            tc.tile_pool(name="dram", bufs=1, space="DRAM") as dramp,
            tc.tile_pool(name="nat", bufs=8) as natp,
            tc.tile_pool(name="natb", bufs=8) as natbp,
            tc.tile_pool(name="qt", bufs=2) as qtp,
            tc.tile_pool(name="extram", bufs=8) as extramp,
            tc.tile_pool(name="sqscr", bufs=3) as sqscrp,
            tc.tile_pool(name="trash", bufs=3) as trashp,
            tc.tile_pool(name="smalls", bufs=4) as smallp,
            tc.tile_pool(name="psum", bufs=2, space="PSUM") as psump,
        ):
            # ---- persistent tiles ----
            statT = singles.tile([128, KT, CORE_ROWS], bf16)   # -2 * xc^T
            sxc = singles.tile([128, TI], f32)
            bias_pos = singles.tile([128, TI], f32)
            bias_g = singles.tile([128, TI], f32)
            pos_acc = singles.tile([128, NSLOTS], f32)
            gjj_acc = singles.tile([128, NSLOTS], f32)
            gjn_acc = singles.tile([128, NSLOTS], f32)
            extraS = singles.tile([EXROWS, TI, 128], bf16)

            nc.gpsimd.dma_start(
                out=extraS[:, :, :],
                in_=exs[:, :].rearrange("c (t i) -> c t i", t=TI))

            # ---- phase 0: stationary = -2 * x_c^T (bf16) + own-row biases ----
            sbf = dramp.tile([CORE_ROWS, D], bf16, tag="sbf")
            for b in range(TI):
                nat = natp.tile([128, D], f32, tag="nat")
                nc.gpsimd.dma_start(out=nat, in_=xc[128 * b:128 * (b + 1), :])
                natb = natbp.tile([128, D], bf16, tag="natb")
                nc.vector.tensor_scalar_mul(out=natb, in0=nat, scalar1=-2.0)
                sq = sqscrp.tile([128, D], f32, tag="sq")
                nc.vector.scalar_tensor_tensor(
                    out=sq, in0=natb, scalar=0.25, in1=natb,
                    op0=Alu.mult, op1=Alu.mult, accum_out=sxc[:, b:b + 1])
                nc.gpsimd.dma_start(out=sbf[128 * b:128 * (b + 1), :], in_=natb)
            for kt in range(KT):
                nc.sync.dma_start_transpose(
                    out=statT[:, kt, :],
                    in_=sbf[:, 128 * kt:128 * (kt + 1)])

            # pos wants relu(psum + a_i - BIG); guard wants relu(-psum + 1 - a_i)
            nc.vector.tensor_scalar(
                out=bias_pos, in0=sxc, scalar1=float(D_EPS2 - BIG),
                scalar2=None, op0=Alu.add)
            nc.vector.tensor_scalar(
                out=bias_g, in0=sxc, scalar1=-1.0,
                scalar2=float(1.0 - D_EPS2), op0=Alu.mult, op1=Alu.add)

            # ---- conversion sweep: fp32 -> bf16 DRAM + |x_j|^2 rows ----
            def convert_quarter(src, mq, qi):
                """Convert 2048 rows to bf16 DRAM; build the [3,16,128] bf16
                hi/mid/lo staged rows of |x_j|^2."""
                qbf = dramp.tile([QCOLS, D], bf16, tag=f"qbf{mq}{qi}")
                qsx = smallp.tile([128, QTILES], f32, tag="qsx")
                for i in range(QTILES):
                    nat = natp.tile([128, D], f32, tag="nat")
                    r0 = QCOLS * qi + 128 * i
                    nc.gpsimd.dma_start(out=nat, in_=src[r0:r0 + 128, :])
                    natb = natbp.tile([128, D], bf16, tag="natb")
                    nc.vector.tensor_copy(out=natb, in_=nat)
                    sq = sqscrp.tile([128, D], f32, tag="sq")
                    nc.vector.scalar_tensor_tensor(
                        out=sq, in0=natb, scalar=1.0, in1=natb,
                        op0=Alu.mult, op1=Alu.mult,
                        accum_out=qsx[:, i:i + 1])
                    nc.gpsimd.dma_start(
                        out=qbf[128 * i:128 * (i + 1), :], in_=natb)
                # 3-way bf16 split of the f32 row sums
                hi = smallp.tile([128, QTILES], bf16, tag="hi")
                mid = smallp.tile([128, QTILES], bf16, tag="mid")
                lo = smallp.tile([128, QTILES], bf16, tag="lo")
                r1 = smallp.tile([128, QTILES], f32, tag="r1")
                r2 = smallp.tile([128, QTILES], f32, tag="r2")
                nc.vector.tensor_copy(out=hi, in_=qsx)
                nc.vector.tensor_tensor(out=r1, in0=qsx, in1=hi,
                                        op=Alu.subtract)
                nc.vector.tensor_copy(out=mid, in_=r1)
                nc.vector.tensor_tensor(out=r2, in0=r1, in1=mid,
                                        op=Alu.subtract)
                nc.vector.tensor_copy(out=lo, in_=r2)
                stg = dramp.tile([3, QTILES, 128], bf16, tag=f"stg{mq}{qi}")
                nc.gpsimd.dma_start(
                    out=stg[0, :, :].rearrange("f p -> p f"), in_=hi)
                nc.gpsimd.dma_start(
                    out=stg[1, :, :].rearrange("f p -> p f"), in_=mid)
                nc.gpsimd.dma_start(
                    out=stg[2, :, :].rearrange("f p -> p f"), in_=lo)
                return qbf, stg

            # ---- main sweep: convert quarter, then immediately compute on
            # it, so engine streams interleave and PSUM drains promptly ----
            for phase, mq, src in (("jj", "x", xfull), ("jn", "y", yfull)):
                for qi in range(NQ):
                    qbf, stg = convert_quarter(src, mq, qi)
                    qt = qtp.tile([128, KT, QCOLS], bf16, tag="qt")
                    for kt in range(KT):
                        nc.sync.dma_start_transpose(
                            out=qt[:, kt, :],
                            in_=qbf[:, 128 * kt:128 * (kt + 1)])
                    ems = []
                    for pq in range(QPANELS):
                        em = extramp.tile([EXROWS, PANEL], bf16, tag="em")
                        nc.gpsimd.dma_start(
                            out=em[0:3, :],
                            in_=stg[:, 4 * pq:4 * (pq + 1), :].rearrange(
                                "c f p -> c (f p)"))
                        if phase == "jj":
                            p = QPANELS * qi + pq
                            nc.gpsimd.dma_start(
                                out=em[3:EXROWS, :],
                                in_=ohb[:, PANEL * p:PANEL * (p + 1)])
                        ems.append(em)
                    for t in range(TI):
                        psum = psump.tile([128, QCOLS], f32, tag="ps")
                        for pq in range(QPANELS):
                            col = slice(PANEL * pq, PANEL * (pq + 1))
                            for kt in range(KT):
                                nc.tensor.matmul(
                                    out=psum[:, col],
                                    lhsT=statT[:, kt, 128 * t:128 * (t + 1)],
                                    rhs=qt[:, kt, col],
                                    start=(kt == 0), stop=False)
                            nrows = EXROWS if phase == "jj" else 3
                            nc.tensor.matmul(
                                out=psum[:, col],
                                lhsT=extraS[0:nrows, t, :],
                                rhs=ems[pq][0:nrows, :],
                                start=False, stop=True)
                        s = t * NQ + qi
                        if phase == "jj":
                            tr = trashp.tile([128, QCOLS], f32, tag="tr")
                            nc.scalar.activation(
                                out=tr, in_=psum, func=Act.Relu,
                                bias=bias_pos[:, t:t + 1], scale=1.0,
                                accum_out=pos_acc[:, s:s + 1])
                            tr2 = trashp.tile([128, QCOLS], f32, tag="tr")
                            nc.scalar.activation(
                                out=tr2, in_=psum, func=Act.Relu,
                                bias=bias_g[:, t:t + 1], scale=-1.0,
                                accum_out=gjj_acc[:, s:s + 1])
                        else:
                            tr = trashp.tile([128, QCOLS], f32, tag="tr")
                            nc.scalar.activation(
                                out=tr, in_=psum, func=Act.Relu,
                                bias=bias_g[:, t:t + 1], scale=-1.0,
                                accum_out=gjn_acc[:, s:s + 1])

            nc.gpsimd.dma_start(out=pos_out[:, :], in_=pos_acc)
            nc.gpsimd.dma_start(out=gjj_out[:, :], in_=gjj_acc)
            nc.gpsimd.dma_start(out=gjn_out[:, :], in_=gjn_acc)

    nc.compile()
    return nc


def _get_program():
    if "nc" not in _CACHE:
        _CACHE["nc"] = _build_program()
    return _CACHE["nc"]


def _host_inputs(joint_embeddings, non_joint_embeddings, joint_labels):
    import ml_dtypes

    x = np.ascontiguousarray(joint_embeddings, dtype=np.float32)
    y = np.ascontiguousarray(non_joint_embeddings, dtype=np.float32)
    lab = np.asarray(joint_labels).astype(np.int64)
    onehot = (lab[None, :] == np.arange(N_LABELS, dtype=np.int64)[:, None])
    ohb = (onehot.astype(np.float32) * np.float32(BIG)).astype(
        ml_dtypes.bfloat16)
    in_maps = []
    for c in range(N_CORES):
        rows = slice(CORE_ROWS * c, CORE_ROWS * (c + 1))
        exs = np.concatenate(
            [np.ones((3, CORE_ROWS), dtype=np.float32),
             onehot[:, rows].astype(np.float32)], axis=0).astype(
                 ml_dtypes.bfloat16)
        in_maps.append({
            "xfull": x, "yfull": y,
            "xc": np.ascontiguousarray(x[rows]),
            "ohb": ohb, "exs": np.ascontiguousarray(exs),
        })
    return in_maps, lab


def _fallback_numpy(x, y, lab):
    """Exact reference evaluation (float64), chunked. Only used when a
    guard fired, i.e. some pair distance is inside the margin."""
    x = x.astype(np.float64)
    y = y.astype(np.float64)
    sx = (x * x).sum(1)
    sy = (y * y).sum(1)
    rx = x.sum(1)
    ry = y.sum(1)
    n = x.shape[0]
    pos_sum = 0.0
    neg_sum = 0.0
    cross_sum = 0.0
    same = lab[:, None] == lab[None, :]
    for i0 in range(0, n, 512):
        i1 = min(i0 + 512, n)
        g = x[i0:i1] @ x.T
        d2 = (sx[i0:i1, None] + sx[None, :] - 2 * g
              + 2 * EPS * (rx[i0:i1, None] - rx[None, :]) + D_EPS2)
        d2 = np.maximum(d2, 0.0)
        upper = np.arange(n)[None, :] > np.arange(i0, i1)[:, None]
        sm = same[i0:i1]
        pos_sum += d2[upper & sm].sum()
        dist = np.sqrt(np.maximum(d2, 1e-12))
        t = np.maximum(MARGIN - dist, 0.0) ** 2
        neg_sum += t[upper & ~sm].sum()
        gy = x[i0:i1] @ y.T
        d2y = (sx[i0:i1, None] + sy[None, :] - 2 * gy
               + 2 * EPS * (rx[i0:i1, None] - ry[None, :]) + D_EPS2)
        d2y = np.maximum(d2y, 0.0)
        disty = np.sqrt(np.maximum(d2y, 1e-12))
        cross_sum += (np.maximum(MARGIN - disty, 0.0) ** 2).sum()
    counts = np.bincount(lab, minlength=N_LABELS)
    n_pos = max(int((counts * (counts - 1) // 2).sum()), 1)
    n_neg = max(n * (n - 1) // 2 - int((counts * (counts - 1) // 2).sum()), 1)
    loss = (pos_sum / n_pos + neg_sum / n_neg
            + cross_sum / (x.shape[0] * y.shape[0]))
    return np.float32(LOSS_WEIGHT * loss)


def kernel(joint_embeddings, non_joint_embeddings, joint_labels):
    from concourse.bass_utils import run_bass_kernel_spmd

    nc = _get_program()
    in_maps, lab = _host_inputs(joint_embeddings, non_joint_embeddings,
                                joint_labels)
    res = run_bass_kernel_spmd(nc, in_maps, core_ids=list(range(N_CORES)))
    _CACHE["last_results"] = res
    return _combine(res.results, joint_embeddings, non_joint_embeddings, lab)


def _combine(results, joint_embeddings, non_joint_embeddings, lab):
    pos_full = 0.0
    guard = 0.0
    for r in results:
        pos_full += float(r["pos_out"].astype(np.float64).sum())
        guard += float(r["gjj_out"].astype(np.float64).sum())
        guard += float(r["gjn_out"].astype(np.float64).sum())
    if guard > 0.0:
        return _fallback_numpy(
            np.asarray(joint_embeddings, dtype=np.float32),
            np.asarray(non_joint_embeddings, dtype=np.float32), lab)
    counts = np.bincount(lab, minlength=N_LABELS)
    n_pos = max(int((counts * (counts - 1) // 2).sum()), 1)
    loss = pos_full / 2.0 / n_pos
    return np.float32(LOSS_WEIGHT * loss)
